# revision 21
# baseline (speedup 1.0000x reference)
"""Trainium2 Bass kernel for multi-head causal attention (v2).

Problem: q, k, v of shape [4096, 16, 64] (seq, heads, head_dim) fp32.
  out = softmax(causal(q @ k^T / 8)) @ v, reshaped to [4096, 1024].

Sharding: heads are split across 8 NeuronCores (2 heads per core).
Each core runs the same SPMD Bass program on its own 2 heads; the host
concatenates the per-core [4096, 128] outputs along the feature dim.

Per-core algorithm (S^T orientation, PE/ACT-balanced design):
  - Inputs stream in as fp32 via the two HWDGE queues (k on sync, q on
    scalar), are cast to bf16 on the DVE, then PE-transposed into
    qT/kT [(h,d)=128, 4096] lazily as each chunk is first needed
    (borrowing main-loop PSUM slots).  V loads via SWDGE cast DMA into
    vplus [128, 32*65]: each 128-row k-block gets 64 V columns plus a
    ones column (fused softmax denominator).
  - q groups (512 wide) are processed HEAVIEST FIRST (G=7..0): the
    causal triangle's thin front would otherwise idle both engines
    for ~20us at the start.  Per G, k blocks j <= 4G+3 in groups of 3;
    the 4 diagonal blocks form one column-packed group (order t0,t1,
    t3,t2 so no matmul output crosses a PSUM bank boundary) so the
    exp never touches masked columns:
      mm1:  S^T[kj, qi] = kT_j^T.T @ qT_G per head (PE row tiling)
      exp:  one ScalarE Exp per (jgroup, head), PSUM -> SBUF bf16;
            heads ping-pong across the two PSUM slots so ACT never
            waits on mm1
      mask: diagonal 128x128 triangle chunks multiplied by a 0/1 mask
      mm2 (reversed): O^T[d(+ones), qi] += vplus_j.T @ expS^T_j —
            V is the 65-column stationary so the PE streams each exp
            block once instead of 4x 128-col LDWEIGHTS per block
  - End of G: copy O^T [65, 512] to SBUF bf16, PE-transpose back into
    the freed PSUM bank ([128, 4*66]: out chunk ++ denominator col),
    reciprocal + row-scale on DVE, one batched output DMA per G.

No distributed primitives are needed: sharding is purely host-side.
"""

import numpy as np

SEQ = 4096
NHEAD = 16
HDIM = 64
NCORES = 8
HPC = NHEAD // NCORES  # heads per core = 2
SCALE = 0.125

_NC_CACHE = {}
LAST_RESULT = {}


def build_attention_nc(seq=SEQ, hpc=HPC, hdim=HDIM):
    """Build the SPMD Bass program for one core handling `hpc` heads."""
    import concourse.bass as bass
    import concourse.mybir as mybir
    import concourse.tile as tile

    f32 = mybir.dt.float32
    bf16 = mybir.dt.bfloat16
    Exp = mybir.ActivationFunctionType.Exp

    assert hpc == 2 and hdim == 64, "layout hardcoded for 2 heads x 64 dim"
    assert seq % 1024 == 0
    nt = seq // 128   # 128-row seq tiles = 32
    ng = seq // 512   # 512-wide q groups = 8
    nchunks = nt // 4  # staging chunks of 4 tiles = 8
    W = 65            # V block width incl ones column

    nc = bass.Bass()
    q = nc.dram_tensor("q", [seq, hpc, hdim], f32, kind="ExternalInput").ap()
    k = nc.dram_tensor("k", [seq, hpc, hdim], f32, kind="ExternalInput").ap()
    v = nc.dram_tensor("v", [seq, hpc, hdim], f32, kind="ExternalInput").ap()
    o = nc.dram_tensor("o", [seq, hpc * hdim], f32, kind="ExternalOutput").ap()

    with tile.TileContext(nc) as tc:
        with (
            tc.tile_pool(name="persist", bufs=1) as persist,
            tc.tile_pool(name="stage", bufs=6) as stage_pool,
            tc.tile_pool(name="pexp", bufs=6) as pexp_pool,
            tc.tile_pool(name="oTs", bufs=2) as oTs_pool,
            tc.tile_pool(name="outp", bufs=2) as out_pool,
            tc.tile_pool(name="small", bufs=8) as small_pool,
            tc.tile_pool(name="psum_s", bufs=2, space="PSUM") as ps_pool,
            tc.tile_pool(name="psum_o", bufs=1, space="PSUM") as po_pool,
        ):
            # ---- persistent SBUF tensors ----------------------------------
            qT = persist.tile([128, seq], bf16, tag="qT")
            kT = persist.tile([128, seq], bf16, tag="kT")
            vplus = [
                persist.tile([128, nt * W], bf16, tag=f"vplus{h}", name=f"vplus{h}")
                for h in range(hpc)
            ]
            # 0/1 lower-triangle mask for the in-chunk diagonal:
            # tri[kj, qi'] = 1 iff kj <= qi'  (same for every diagonal block)
            tri = persist.tile([128, 128], bf16, tag="tri")
            ident_b = persist.tile([128, 128], bf16, tag="identb")

            from concourse.masks import make_identity

            # ---- K/Q loads: HWDGE fp32 (k on sync, q on scalar) + DVE
            # cast to bf16 staging.  k chunks ascend (G descends but its
            # j-loop ascends); q chunks descend to match the G order.
            stage_k, stage_q = [None] * nchunks, [None] * nchunks

            def load_chunk(src_t, eng, lst, c):
                st = stage_pool.tile([128, 4 * 128], f32, tag="st", name="st")
                eng.dma_start(
                    out=st.rearrange("p (t x) -> p t x", x=128),
                    in_=src_t[c * 512 : (c + 1) * 512, :, :].rearrange(
                        "(t p) h d -> p t (h d)", p=128
                    ),
                )
                # one slot per chunk: the input DMAs are queued ahead of
                # all exp work on the HWDGE queues, so slot reuse here
                # would deadlock against main-loop progress
                stb = stage_pool.tile(
                    [128, 4 * 128], bf16, tag="stb", name="stb", bufs=16
                )
                nc.vector.tensor_copy(stb, st)
                lst[c] = stb

            load_chunk(k, nc.sync, stage_k, 0)
            load_chunk(q, nc.scalar, stage_q, nchunks - 1)

            # Preload the exp table + warm the PE clock while the first
            # DMAs are in flight: a dummy 1-col exp pulls the ~1.3us
            # ACT_TABLE_LOAD off the critical path, and ~3us of dummy
            # matmuls into the (idle until mm2) po slot release the HAM
            # clock gate (1.2 -> 2.4 GHz) before real PE work arrives.
            wsrc = persist.tile([128, 128], bf16, tag="wsrc", name="wsrc")
            wout = persist.tile([128, 1], bf16, tag="wout", name="wout")
            nc.vector.memset(wsrc, 0.0)
            nc.scalar.activation(
                out=wout, in_=wsrc[:, 0:1], func=Exp, scale=1.0
            )
            wps = po_pool.tile([128, 512], f32, tag="po0", name="wps")
            for _ in range(28):
                nc.tensor.matmul(
                    wps[:, 0:128], lhsT=wsrc, rhs=wsrc,
                    start=True, stop=True, skip_group_check=True,
                )

            for h in range(hpc):
                nc.vector.memset(vplus[h], 1.0)
                nc.gpsimd.dma_start(
                    out=vplus[h].rearrange("p (t x) -> p t x", x=W)[:, :, 0:hdim],
                    in_=v[:, h, :].rearrange("(t p) d -> p t d", p=128),
                )
            for c in range(1, nchunks):
                load_chunk(k, nc.sync, stage_k, c)
                load_chunk(q, nc.scalar, stage_q, nchunks - 1 - c)

            make_identity(nc, ident_b[:])
            nc.vector.memset(tri, 1.0)
            nc.gpsimd.affine_select(
                out=tri[:],
                in_=tri[:],
                compare_op=mybir.AluOpType.is_ge,
                fill=0.0,
                base=0,
                pattern=[[1, 128]],
                channel_multiplier=-1,
            )

            # ---- main pipeline --------------------------------------------
            # Packed diagonal col offsets/widths.  Order t0,t1,t3,t2 keeps
            # every block inside one 512-col PSUM bank (matmul outputs
            # must not cross bank boundaries) with no unwritten gaps.
            DOFF = [0, 512, 1024, 896]
            DW = [512, 384, 256, 128]

            def tr_batch(st, dstT, c):
                """Transpose 8 staged bf16 seq-tiles into dstT columns,
                borrowing one main-loop PSUM slot as scratch."""
                trs = ps_pool.tile([128, 1536], bf16, tag="ps", name="trs")
                for t in range(4):
                    nc.tensor.transpose(
                        trs[:, t * 128 : (t + 1) * 128],
                        st[:, t * 128 : (t + 1) * 128],
                        ident_b[:],
                    )
                nc.vector.tensor_copy(
                    dstT[:, c * 512 : (c + 1) * 512], trs[:, 0:512]
                )

            def emit_mm2s(pend):
                """Deferred reversed-orientation P@V for one jgroup."""
                G, items, po, pes, njs = pend[:5]
                for h in range(hpc):
                    for j, moff, width, qcol in items:
                        nc.tensor.matmul(
                            po[h][:, qcol : qcol + width],
                            lhsT=vplus[h][:, j * W : (j + 1) * W],
                            rhs=pes[h][:, moff : moff + width],
                            start=(j == 0),
                            stop=(j == njs - 1),
                            skip_group_check=True,
                        )

            def emit_finals(G, po):
                oTs = []
                for h in range(hpc):
                    ot = oTs_pool.tile([W, 512], bf16, tag="oTs", name="oTs")
                    nc.vector.tensor_copy(ot, po[h][:, :])
                    oTs.append(ot)
                WP = W + 1  # pad chunk stride to 66 cols = 132 B (4B-aligned)
                trh, recs = [], []
                for h in range(hpc):
                    trp = po_pool.tile(
                        [128, 4 * WP], bf16, tag=f"po{h}", name=f"tr{h}"
                    )
                    for cc in range(4):
                        nc.tensor.transpose(
                            trp[:, cc * WP : cc * WP + W],
                            oTs[h][:, cc * 128 : (cc + 1) * 128],
                            ident_b[0:W, 0:W],
                        )
                    rec = small_pool.tile([128, 4], f32, tag="rec", name="rec")
                    nc.vector.reciprocal(
                        rec, trp.rearrange("p (c x) -> p c x", x=WP)[:, :, hdim]
                    )
                    trh.append(trp)
                    recs.append(rec)
                ob = out_pool.tile([128, 4 * hpc * hdim], f32, tag="ob", name="ob")
                for cc in range(4):
                    for h in range(hpc):
                        nc.vector.tensor_scalar_mul(
                            ob[:, cc * 128 + h * hdim : cc * 128 + (h + 1) * hdim],
                            trh[h][:, cc * WP : cc * WP + hdim],
                            recs[h][:, cc : cc + 1],
                        )
                nc.sync.dma_start(
                    out=o[G * 512 : (G + 1) * 512, :].rearrange(
                        "(c p) d -> p c d", p=128
                    ),
                    in_=ob.rearrange("p (c d) -> p c d", d=hpc * hdim),
                )

            k_tr_done, q_tr_done = set(), set()

            def ensure_tr(done, stages, dstT, c):
                if c not in done:
                    done.add(c)
                    tr_batch(stages[c], dstT, c)

            # Heaviest groups first: the causal triangle's thin front
            # (G0..G2) otherwise idles both engines at the start.
            pending = None
            for G in reversed(range(ng)):
                njs = 4 * G + 4
                po = [
                    po_pool.tile([W, 512], f32, tag=f"po{h}", name=f"po{h}")
                    for h in range(hpc)
                ]
                ensure_tr(q_tr_done, stage_q, qT, G)

                # jgroups: off-diagonal in threes, diagonal packed
                jgroups = [
                    [(j, (j - s) * 512, 512, 0) for j in range(s, min(s + 3, 4 * G))]
                    for s in range(0, 4 * G, 3)
                ]
                jgroups.append(
                    [(4 * G + t, DOFF[t], DW[t], 128 * t) for t in range(4)]
                )
                for gi, items in enumerate(jgroups):
                    ensure_tr(k_tr_done, stage_k, kT, items[-1][0] // 4)
                    width = max(moff + wd for _, moff, wd, _ in items)
                    ps = [
                        ps_pool.tile([128, 1536], f32, tag="ps", name="ps")
                        for _ in range(hpc)
                    ]
                    for j, moff, wd, qcol in items:
                        for h in range(hpc):
                            nc.tensor.matmul(
                                ps[h][:, moff : moff + wd],
                                lhsT=kT[h * 64 : (h + 1) * 64, j * 128 : (j + 1) * 128],
                                rhs=qT[
                                    h * 64 : (h + 1) * 64,
                                    G * 512 + qcol : (G + 1) * 512,
                                ],
                                start=True,
                                stop=True,
                                tile_position=(h * 64, 0),
                            )
                    pes = []
                    for h in range(hpc):
                        pe = pexp_pool.tile([128, 1536], bf16, tag="pexp", name="pexp")
                        nc.scalar.activation(
                            out=pe[:, 0:width],
                            in_=ps[h][:, 0:width],
                            func=Exp,
                            scale=SCALE,
                        )
                        pes.append(pe)
                    if gi == len(jgroups) - 1:  # diagonal group: triangle masks
                        for h in range(hpc):
                            for t in range(4):
                                nc.vector.tensor_mul(
                                    pes[h][:, DOFF[t] : DOFF[t] + 128],
                                    pes[h][:, DOFF[t] : DOFF[t] + 128],
                                    tri[:],
                                )
                    if pending is not None:
                        emit_mm2s(pending)
                        if pending[5]:
                            emit_finals(pending[0], pending[2])
                    pending = (G, items, po, pes, njs, gi == len(jgroups) - 1)
            if pending is not None:
                emit_mm2s(pending)
                emit_finals(pending[0], pending[2])
    _split_multi_waits(nc)
    return nc


def _split_multi_waits(nc):
    """Walrus's codegen accepts at most one sync-wait per instruction on
    this toolchain. Hoist extra waits into standalone single-wait NoOps on
    the same engine queue (same semantics: the sequencer stalls in order)."""
    import concourse.mybir as mybir

    nsplit = 0
    for blk in nc.m.functions[0].blocks:
        newl = []
        for ins in blk.instructions:
            si = getattr(ins, "sync_info", None)
            if si is not None and si.on_wait and len(si.on_wait) > 1:
                waits = list(si.on_wait)
                for w in waits[:-1]:
                    newl.append(
                        mybir.InstNoOp(
                            name=f"{ins.name}-wsplit{nsplit}",
                            sync_info=mybir.SyncInfo(on_wait=[w], on_update=[]),
                            bass_nofuse=True,
                            engine=ins.engine,
                            ins=[],
                            outs=[],
                        )
                    )
                    nsplit += 1
                ins.sync_info = mybir.SyncInfo(
                    on_wait=[waits[-1]], on_update=list(si.on_update or [])
                )
            newl.append(ins)
        blk.instructions = newl
    return nsplit


def _ensure_ntff_hook():
    """The image's antenv package lacks axon_hooks; provide it so
    run_bass_kernel_spmd's trace path works (or degrades gracefully)."""
    import sys
    import types

    try:
        import antenv.axon_hooks  # noqa: F401

        return
    except ImportError:
        pass
    mod = types.ModuleType("antenv.axon_hooks")
    state = {"hook": None}
    mod.set_axon_ntff_profile_hook = lambda h: state.__setitem__("hook", h)
    mod.get_axon_ntff_profile_hook = lambda: state["hook"]
    try:
        from trn_agent_boot.trn_boot import _ntff_profile_via_ctypes

        state["hook"] = _ntff_profile_via_ctypes("/opt/axon/libaxon_pjrt.so")
    except Exception:
        state["hook"] = None
    sys.modules["antenv.axon_hooks"] = mod


def kernel(q, k, v):
    """Full-input entry point: q, k, v [4096, 16, 64] fp32 -> [4096, 1024]."""
    import sys

    if "/opt/trn_rl_repo" not in sys.path:
        sys.path.insert(0, "/opt/trn_rl_repo")
    _ensure_ntff_hook()
    from concourse.bass_utils import run_bass_kernel_spmd

    q = np.asarray(q, dtype=np.float32)
    k = np.asarray(k, dtype=np.float32)
    v = np.asarray(v, dtype=np.float32)
    seq, nhead, hdim = q.shape

    if "nc" not in _NC_CACHE:
        _NC_CACHE["nc"] = build_attention_nc(seq=seq, hpc=HPC, hdim=hdim)
    nc = _NC_CACHE["nc"]

    in_maps = []
    for c in range(NCORES):
        hs = slice(c * HPC, (c + 1) * HPC)
        in_maps.append(
            {
                "q": np.ascontiguousarray(q[:, hs, :]),
                "k": np.ascontiguousarray(k[:, hs, :]),
                "v": np.ascontiguousarray(v[:, hs, :]),
            }
        )
    res = run_bass_kernel_spmd(nc, in_maps, core_ids=list(range(NCORES)))
    LAST_RESULT["exec_time_ns"] = res.exec_time_ns
    try:
        iat = res.instructions_and_trace
        LAST_RESULT["trace_path"] = iat[1] if iat else None
    except Exception:
        LAST_RESULT["trace_path"] = None
    outs = [res.results[c]["o"] for c in range(NCORES)]
    return np.concatenate(outs, axis=1)


# revision 23
# speedup vs baseline: 1.1505x; 1.1505x over previous
"""Trainium2 Bass kernel for multi-head causal attention (v2).

Problem: q, k, v of shape [4096, 16, 64] (seq, heads, head_dim) fp32.
  out = softmax(causal(q @ k^T / 8)) @ v, reshaped to [4096, 1024].

Sharding: heads are split across 8 NeuronCores (2 heads per core).
Each core runs the same SPMD Bass program on its own 2 heads; the host
concatenates the per-core [4096, 128] outputs along the feature dim.

Per-core algorithm (S^T orientation, PE/ACT-balanced design):
  - Inputs stream in as fp32 via the two HWDGE queues (k on sync, q on
    scalar), are cast to bf16 on the DVE, then PE-transposed into
    qT/kT [(h,d)=128, 4096] lazily as each chunk is first needed
    (borrowing main-loop PSUM slots).  V loads via SWDGE cast DMA into
    vplus [128, 32*65]: each 128-row k-block gets 64 V columns plus a
    ones column (fused softmax denominator).
  - q groups (512 wide) are processed HEAVIEST FIRST (G=7..0): the
    causal triangle's thin front would otherwise idle both engines
    for ~20us at the start.  Per G, k blocks j <= 4G+3 in groups of 3;
    the 4 diagonal blocks form one column-packed group (order t0,t1,
    t3,t2 so no matmul output crosses a PSUM bank boundary) so the
    exp never touches masked columns:
      mm1:  S^T[kj, qi] = kT_j^T.T @ qT_G per head (PE row tiling)
      exp:  one ScalarE Exp per (jgroup, head), PSUM -> SBUF bf16;
            heads ping-pong across the two PSUM slots so ACT never
            waits on mm1
      mask: diagonal 128x128 triangle chunks multiplied by a 0/1 mask
      mm2 (reversed): O^T[d(+ones), qi] += vplus_j.T @ expS^T_j —
            V is the 65-column stationary so the PE streams each exp
            block once instead of 4x 128-col LDWEIGHTS per block
  - End of G: copy O^T [65, 512] to SBUF bf16, PE-transpose back into
    the freed PSUM bank ([128, 4*66]: out chunk ++ denominator col),
    reciprocal + row-scale on DVE, one batched output DMA per G.

No distributed primitives are needed: sharding is purely host-side.
"""

import numpy as np

SEQ = 4096
NHEAD = 16
HDIM = 64
NCORES = 8
HPC = NHEAD // NCORES  # heads per core = 2
SCALE = 0.125

_NC_CACHE = {}
LAST_RESULT = {}


def build_attention_nc(seq=SEQ, hpc=HPC, hdim=HDIM):
    """Build the SPMD Bass program for one core handling `hpc` heads."""
    import concourse.bass as bass
    import concourse.mybir as mybir
    import concourse.tile as tile

    f32 = mybir.dt.float32
    bf16 = mybir.dt.bfloat16
    Exp = mybir.ActivationFunctionType.Exp

    assert hpc == 2 and hdim == 64, "layout hardcoded for 2 heads x 64 dim"
    assert seq % 1024 == 0
    nt = seq // 128   # 128-row seq tiles = 32
    ng = seq // 512   # 512-wide q groups = 8
    nchunks = nt // 8  # staging chunks of 8 tiles = 4
    W = 65            # V block width incl ones column

    nc = bass.Bass()
    q = nc.dram_tensor("q", [seq, hpc, hdim], f32, kind="ExternalInput").ap()
    k = nc.dram_tensor("k", [seq, hpc, hdim], f32, kind="ExternalInput").ap()
    v = nc.dram_tensor("v", [seq, hpc, hdim], f32, kind="ExternalInput").ap()
    o = nc.dram_tensor("o", [seq, hpc * hdim], f32, kind="ExternalOutput").ap()

    with tile.TileContext(nc) as tc:
        with (
            tc.tile_pool(name="persist", bufs=1) as persist,
            tc.tile_pool(name="stage", bufs=6) as stage_pool,
            tc.tile_pool(name="pexp", bufs=6) as pexp_pool,
            tc.tile_pool(name="oTs", bufs=2) as oTs_pool,
            tc.tile_pool(name="outp", bufs=2) as out_pool,
            tc.tile_pool(name="small", bufs=8) as small_pool,
            tc.tile_pool(name="psum_s", bufs=2, space="PSUM") as ps_pool,
            tc.tile_pool(name="psum_o", bufs=1, space="PSUM") as po_pool,
        ):
            # ---- persistent SBUF tensors ----------------------------------
            qT = persist.tile([128, seq], bf16, tag="qT")
            kT = persist.tile([128, seq], bf16, tag="kT")
            vplus = [
                persist.tile([128, nt * W], bf16, tag=f"vplus{h}", name=f"vplus{h}")
                for h in range(hpc)
            ]
            # 0/1 lower-triangle mask for the in-chunk diagonal:
            # tri[kj, qi'] = 1 iff kj <= qi'  (same for every diagonal block)
            tri = persist.tile([128, 128], bf16, tag="tri")
            ident_b = persist.tile([128, 128], bf16, tag="identb")

            from concourse.masks import make_identity

            # ---- K/Q loads: HWDGE fp32 (k on sync, q on scalar) + DVE
            # cast to bf16 staging.  k chunks ascend (G descends but its
            # j-loop ascends); q chunks descend to match the G order.
            stage_k, stage_q = [None] * nchunks, [None] * nchunks

            def load_chunk(src_t, eng, lst, c):
                st = stage_pool.tile([128, 8 * 128], f32, tag="st", name="st")
                eng.dma_start(
                    out=st.rearrange("p (t x) -> p t x", x=128),
                    in_=src_t[c * 1024 : (c + 1) * 1024, :, :].rearrange(
                        "(t p) h d -> p t (h d)", p=128
                    ),
                )
                # one slot per chunk: the input DMAs are queued ahead of
                # all exp work on the HWDGE queues, so slot reuse here
                # would deadlock against main-loop progress
                stb = stage_pool.tile(
                    [128, 8 * 128], bf16, tag="stb", name="stb", bufs=8
                )
                nc.vector.tensor_copy(stb, st)
                lst[c] = stb

            # Preload the exp table + warm the PE clock while the first
            # DMAs are in flight: a dummy 1-col exp pulls the ~1.3us
            # ACT_TABLE_LOAD off the critical path, and ~3us of dummy
            # matmuls into the (idle until mm2) po slot release the HAM
            # clock gate (1.2 -> 2.4 GHz) before real PE work arrives.
            # Emitted FIRST so the DVE memset isn't queued behind the
            # DMA-waiting casts on the in-order vector queue.
            wsrc = persist.tile([128, 128], bf16, tag="wsrc", name="wsrc")
            wout = persist.tile([128, 1], bf16, tag="wout", name="wout")
            nc.vector.memset(wsrc, 0.0)
            nc.scalar.activation(
                out=wout, in_=wsrc[:, 0:1], func=Exp, scale=1.0
            )
            wps = po_pool.tile([128, 512], f32, tag="po0", name="wps")
            for _ in range(28):
                nc.tensor.matmul(
                    wps[:, 0:128], lhsT=wsrc, rhs=wsrc,
                    start=True, stop=True, skip_group_check=True,
                )

            load_chunk(k, nc.sync, stage_k, 0)
            load_chunk(q, nc.scalar, stage_q, nchunks - 1)
            for h in range(hpc):
                nc.vector.memset(vplus[h], 1.0)
                nc.gpsimd.dma_start(
                    out=vplus[h].rearrange("p (t x) -> p t x", x=W)[:, :, 0:hdim],
                    in_=v[:, h, :].rearrange("(t p) d -> p t d", p=128),
                )
            for c in range(1, nchunks):
                load_chunk(k, nc.sync, stage_k, c)
                load_chunk(q, nc.scalar, stage_q, nchunks - 1 - c)

            make_identity(nc, ident_b[:])
            nc.vector.memset(tri, 1.0)
            nc.gpsimd.affine_select(
                out=tri[:],
                in_=tri[:],
                compare_op=mybir.AluOpType.is_ge,
                fill=0.0,
                base=0,
                pattern=[[1, 128]],
                channel_multiplier=-1,
            )

            # ---- main pipeline --------------------------------------------
            # Packed diagonal col offsets/widths.  Order t0,t1,t3,t2 keeps
            # every block inside one 512-col PSUM bank (matmul outputs
            # must not cross bank boundaries) with no unwritten gaps.
            DOFF = [0, 512, 1024, 896]
            DW = [512, 384, 256, 128]

            def tr_batch(st, dstT, c):
                """Transpose 8 staged bf16 seq-tiles into dstT columns,
                borrowing one main-loop PSUM slot as scratch."""
                trs = ps_pool.tile([128, 1536], bf16, tag="ps", name="trs")
                for t in range(8):
                    nc.tensor.transpose(
                        trs[:, t * 128 : (t + 1) * 128],
                        st[:, t * 128 : (t + 1) * 128],
                        ident_b[:],
                    )
                nc.vector.tensor_copy(
                    dstT[:, c * 1024 : (c + 1) * 1024], trs[:, 0:1024]
                )

            def emit_mm2s(pend):
                """Deferred reversed-orientation P@V for one jgroup."""
                G, items, po, pes, njs = pend[:5]
                for h in range(hpc):
                    for j, moff, width, qcol in items:
                        nc.tensor.matmul(
                            po[h][:, qcol : qcol + width],
                            lhsT=vplus[h][:, j * W : (j + 1) * W],
                            rhs=pes[h][:, moff : moff + width],
                            start=(j == 0),
                            stop=(j == njs - 1),
                            skip_group_check=True,
                        )

            def emit_finals(G, po):
                oTs = []
                for h in range(hpc):
                    ot = oTs_pool.tile([W, 512], bf16, tag="oTs", name="oTs")
                    nc.vector.tensor_copy(ot, po[h][:, :])
                    oTs.append(ot)
                WP = W + 1  # pad chunk stride to 66 cols = 132 B (4B-aligned)
                trh, recs = [], []
                for h in range(hpc):
                    trp = po_pool.tile(
                        [128, 4 * WP], bf16, tag=f"po{h}", name=f"tr{h}"
                    )
                    for cc in range(4):
                        nc.tensor.transpose(
                            trp[:, cc * WP : cc * WP + W],
                            oTs[h][:, cc * 128 : (cc + 1) * 128],
                            ident_b[0:W, 0:W],
                        )
                    rec = small_pool.tile([128, 4], f32, tag="rec", name="rec")
                    nc.vector.reciprocal(
                        rec, trp.rearrange("p (c x) -> p c x", x=WP)[:, :, hdim]
                    )
                    trh.append(trp)
                    recs.append(rec)
                ob = out_pool.tile([128, 4 * hpc * hdim], f32, tag="ob", name="ob")
                for cc in range(4):
                    for h in range(hpc):
                        nc.vector.tensor_scalar_mul(
                            ob[:, cc * 128 + h * hdim : cc * 128 + (h + 1) * hdim],
                            trh[h][:, cc * WP : cc * WP + hdim],
                            recs[h][:, cc : cc + 1],
                        )
                nc.sync.dma_start(
                    out=o[G * 512 : (G + 1) * 512, :].rearrange(
                        "(c p) d -> p c d", p=128
                    ),
                    in_=ob.rearrange("p (c d) -> p c d", d=hpc * hdim),
                )

            k_tr_done, q_tr_done = set(), set()

            def ensure_tr(done, stages, dstT, c):
                if c not in done:
                    done.add(c)
                    tr_batch(stages[c], dstT, c)

            # Heaviest groups first: the causal triangle's thin front
            # (G0..G2) otherwise idles both engines at the start.
            pending = None
            for G in reversed(range(ng)):
                njs = 4 * G + 4
                po = [
                    po_pool.tile([W, 512], f32, tag=f"po{h}", name=f"po{h}")
                    for h in range(hpc)
                ]
                ensure_tr(q_tr_done, stage_q, qT, G // 2)

                # jgroups: off-diagonal in threes, diagonal packed
                jgroups = [
                    [(j, (j - s) * 512, 512, 0) for j in range(s, min(s + 3, 4 * G))]
                    for s in range(0, 4 * G, 3)
                ]
                jgroups.append(
                    [(4 * G + t, DOFF[t], DW[t], 128 * t) for t in range(4)]
                )
                for gi, items in enumerate(jgroups):
                    ensure_tr(k_tr_done, stage_k, kT, items[-1][0] // 8)
                    width = max(moff + wd for _, moff, wd, _ in items)
                    ps = [
                        ps_pool.tile([128, 1536], f32, tag="ps", name="ps")
                        for _ in range(hpc)
                    ]
                    for j, moff, wd, qcol in items:
                        for h in range(hpc):
                            nc.tensor.matmul(
                                ps[h][:, moff : moff + wd],
                                lhsT=kT[h * 64 : (h + 1) * 64, j * 128 : (j + 1) * 128],
                                rhs=qT[
                                    h * 64 : (h + 1) * 64,
                                    G * 512 + qcol : (G + 1) * 512,
                                ],
                                start=True,
                                stop=True,
                                tile_position=(h * 64, 0),
                            )
                    pes = []
                    for h in range(hpc):
                        pe = pexp_pool.tile([128, 1536], bf16, tag="pexp", name="pexp")
                        nc.scalar.activation(
                            out=pe[:, 0:width],
                            in_=ps[h][:, 0:width],
                            func=Exp,
                            scale=SCALE,
                        )
                        pes.append(pe)
                    if gi == len(jgroups) - 1:  # diagonal group: triangle masks
                        for h in range(hpc):
                            for t in range(4):
                                nc.vector.tensor_mul(
                                    pes[h][:, DOFF[t] : DOFF[t] + 128],
                                    pes[h][:, DOFF[t] : DOFF[t] + 128],
                                    tri[:],
                                )
                    if pending is not None:
                        emit_mm2s(pending)
                        if pending[5]:
                            emit_finals(pending[0], pending[2])
                    pending = (G, items, po, pes, njs, gi == len(jgroups) - 1)
            if pending is not None:
                emit_mm2s(pending)
                emit_finals(pending[0], pending[2])
    _split_multi_waits(nc)
    return nc


def _split_multi_waits(nc):
    """Walrus's codegen accepts at most one sync-wait per instruction on
    this toolchain. Hoist extra waits into standalone single-wait NoOps on
    the same engine queue (same semantics: the sequencer stalls in order)."""
    import concourse.mybir as mybir

    nsplit = 0
    for blk in nc.m.functions[0].blocks:
        newl = []
        for ins in blk.instructions:
            si = getattr(ins, "sync_info", None)
            if si is not None and si.on_wait and len(si.on_wait) > 1:
                waits = list(si.on_wait)
                for w in waits[:-1]:
                    newl.append(
                        mybir.InstNoOp(
                            name=f"{ins.name}-wsplit{nsplit}",
                            sync_info=mybir.SyncInfo(on_wait=[w], on_update=[]),
                            bass_nofuse=True,
                            engine=ins.engine,
                            ins=[],
                            outs=[],
                        )
                    )
                    nsplit += 1
                ins.sync_info = mybir.SyncInfo(
                    on_wait=[waits[-1]], on_update=list(si.on_update or [])
                )
            newl.append(ins)
        blk.instructions = newl
    return nsplit


def _ensure_ntff_hook():
    """The image's antenv package lacks axon_hooks; provide it so
    run_bass_kernel_spmd's trace path works (or degrades gracefully)."""
    import sys
    import types

    try:
        import antenv.axon_hooks  # noqa: F401

        return
    except ImportError:
        pass
    mod = types.ModuleType("antenv.axon_hooks")
    state = {"hook": None}
    mod.set_axon_ntff_profile_hook = lambda h: state.__setitem__("hook", h)
    mod.get_axon_ntff_profile_hook = lambda: state["hook"]
    try:
        from trn_agent_boot.trn_boot import _ntff_profile_via_ctypes

        state["hook"] = _ntff_profile_via_ctypes("/opt/axon/libaxon_pjrt.so")
    except Exception:
        state["hook"] = None
    sys.modules["antenv.axon_hooks"] = mod


def kernel(q, k, v):
    """Full-input entry point: q, k, v [4096, 16, 64] fp32 -> [4096, 1024]."""
    import sys

    if "/opt/trn_rl_repo" not in sys.path:
        sys.path.insert(0, "/opt/trn_rl_repo")
    _ensure_ntff_hook()
    from concourse.bass_utils import run_bass_kernel_spmd

    q = np.asarray(q, dtype=np.float32)
    k = np.asarray(k, dtype=np.float32)
    v = np.asarray(v, dtype=np.float32)
    seq, nhead, hdim = q.shape

    if "nc" not in _NC_CACHE:
        _NC_CACHE["nc"] = build_attention_nc(seq=seq, hpc=HPC, hdim=hdim)
    nc = _NC_CACHE["nc"]

    in_maps = []
    for c in range(NCORES):
        hs = slice(c * HPC, (c + 1) * HPC)
        in_maps.append(
            {
                "q": np.ascontiguousarray(q[:, hs, :]),
                "k": np.ascontiguousarray(k[:, hs, :]),
                "v": np.ascontiguousarray(v[:, hs, :]),
            }
        )
    res = run_bass_kernel_spmd(nc, in_maps, core_ids=list(range(NCORES)))
    LAST_RESULT["exec_time_ns"] = res.exec_time_ns
    try:
        iat = res.instructions_and_trace
        LAST_RESULT["trace_path"] = iat[1] if iat else None
    except Exception:
        LAST_RESULT["trace_path"] = None
    outs = [res.results[c]["o"] for c in range(NCORES)]
    return np.concatenate(outs, axis=1)
